# revision 83
# baseline (speedup 1.0000x reference)
"""MentionScore fused Bass kernel for 8 Trainium2 NeuronCores.

Strategy (self-contained, hardcoded for the nn_MentionScore problem):
  - Spans are bucketed by start//6250 -> one bucket per core; each core only
    needs its 6250-token slice (+9 halo), so states/embeds are sharded with
    no collectives.
  - Token phase (feature-major): per-token attention-logit MLP -> e = exp(a);
    the span MLP's first layer is decomposed through the gathers:
        g @ sw1 = A[start] + B[end] + (pooled @ sw1_p) + width-term
    with A = states@sw1[0:400], B = states@sw1[400:800], and
    pooled @ sw1_p = (P[end] - P[start-1]) / (Pe[end] - Pe[start-1]) where
    [P | Pe] = inclusive prefix sums of [e*EC | e], EC = embeds@sw1[800:1150].
    Prefix sums run on the vector engine (tensor_tensor_scan along tokens).
    The token loop is software-pipelined: block b's table export (PE
    transposes + PSUM->SBUF copies + table DMAs) is emitted after block
    b+1's compute stage so the PE queue never head-blocks on the scans.
  - Tables are token-major with 2048B-stride rows laid out
        tabX[t] = [A[t] (150 bf16) | C[t-1] (151 f32) | B[t-1] (150 bf16)]
    so one batched SWDGE dma_gather window [0:512) at `start` yields
    A[start] + the exclusive prefix, and a second window [150:662) at
    `end+1` yields the inclusive prefix + B[end] — C is written once and
    shared by both windows (994ns SWDGE fixed cost paid per chunk instead
    of per 128-span tile; elem_step=1024 strides the 512-elem windows).
  - Tables come in three stages (token ranges [0,2560), [2560,5120), rest).
    Spans are bucketed by `end`; since width <= 10 both gather indices of a
    span live in its stage's range (+small sliver), so each table row is
    written exactly once and span chunks fire as soon as their stage's
    token blocks are exported.
  - Span phase is emitted stage-batched over groups of 4 tiles (512 spans):
    per-tile ops are fused (scalar_tensor_tensor joins the pooled-term
    normalize with the PSUM accumulator) and layer-2/3 matmuls + ReLUs run
    512 wide, so each engine queue streams instead of ping-ponging.
"""

import ml_dtypes
import numpy as np

BF16NP = ml_dtypes.bfloat16

# ---- problem constants (hardcoded per contract) ----
T, S = 50000, 100000
DS, DE, H, DW = 400, 350, 150, 20
W_MAX = 10
BINS5 = np.array([1, 2, 3, 4, 8], np.int64)
NCORES = 8
TPC = T // NCORES            # 6250 tokens per core bucket
TL_PAD = 6272                # 49 * 128 padded local tokens (6250 + 9 halo -> 6259)
NT_S = 102                   # span tiles per core
SMAX = NT_S * 128            # 14336 padded spans per core
TOK_BLOCKS = [(0, 128), (128, 384)] + [(i * 512, 512) for i in range(1, 12)] + [(6144, 128)]
# span stage buckets by end-token: [0,2560) / [2560,5120) / [5120,...)
SPLIT1, SPLIT2 = 2560, 5120
NT1, NT2, NT3 = 42, 41, 19   # span tiles per stage (max observed 5253/5174/2373)
OFF2, OFF3 = 2528, 5088      # table row offsets (32-aligned slivers)
ROWS1, ROWS2, ROWS3 = 2576, 2624, 1312
K400 = [(0, 128), (128, 256), (256, 384), (384, 400)]
K350 = [(0, 128), (128, 256), (256, 350)]
K150 = [(0, 128), (128, 150)]
# packed weight wall: [wk4 x4 | wk3 x3 | wk1 x2 | dtab | bias(16 bf16-cols)]
WCOLS = [480] * 4 + [150] * 3 + [302] * 2 + [150] + [16]
WOFF = np.concatenate([[0], np.cumsum(WCOLS)]).astype(int)
WTOT = int(WOFF[-1])

_PROGRAM_CACHE = {}


def _build_program():
    import concourse.bacc as bacc
    import concourse.bass as bass
    import concourse.mybir as mybir
    import concourse.tile as tile
    from concourse.masks import make_identity

    F32 = mybir.dt.float32
    BF16 = mybir.dt.bfloat16
    I16 = mybir.dt.int16
    AF = mybir.ActivationFunctionType
    OP = mybir.AluOpType

    nc = bacc.Bacc("TRN2", num_devices=NCORES)

    # ---- I/O ----
    # packed [states.T (400, pad to 512) | embeds.T (350, pad to 384)] rows
    seTd = nc.dram_tensor("seT", [896, TL_PAD], BF16, kind="ExternalInput")
    walld = nc.dram_tensor("wall", [128, WTOT], BF16, kind="ExternalInput")
    idxd = nc.dram_tensor("idx2", [128, 2 * (SMAX // 16)], I16, kind="ExternalInput")
    mhd = nc.dram_tensor("mh", [5, SMAX], BF16, kind="ExternalInput")
    scoresd = nc.dram_tensor("scores", [1, SMAX], F32, kind="ExternalOutput")

    tabs = [nc.dram_tensor(f"tabX_{i}", [rows, 1024], BF16)
            for i, rows in enumerate([ROWS1, ROWS2, ROWS3])]

    with tile.TileContext(nc) as tc:
        with (
            tc.tile_pool(name="wpool", bufs=1) as wp,
            tc.tile_pool(name="tok", bufs=3) as tok,
            tc.tile_pool(name="sp4", bufs=3) as sp4,
            tc.tile_pool(name="sp2", bufs=2) as sp2,
            tc.tile_pool(name="sp3", bufs=3) as sp3,
            tc.tile_pool(name="gat", bufs=2) as gp,
            tc.tile_pool(name="ps", bufs=1, space="PSUM") as ps,
        ):
            # ---- resident weights / constants (one DMA for the wall) ----
            wall = wp.tile([128, WTOT], BF16, name="wall")
            nc.sync.dma_start(wall[:], walld[:, :])
            wk4 = [wall[0:k1 - k0, WOFF[i]:WOFF[i] + 480] for i, (k0, k1) in enumerate(K400)]
            wk3 = [wall[0:k1 - k0, WOFF[4 + i]:WOFF[4 + i] + 150] for i, (k0, k1) in enumerate(K350)]
            wk1 = [wall[0:k1 - k0, WOFF[7 + i]:WOFF[7 + i] + 302] for i, (k0, k1) in enumerate(K150)]
            w_dt = wall[0:5, WOFF[9]:WOFF[9] + 150]
            b1 = wall[:, WOFF[10]:WOFF[10] + 16].bitcast(F32)

            w_aw1 = [w[:, 0:128] for w in wk4]
            w_sa = [w[:, 128:256] for w in wk4]
            w_sb = [w[:, 256:384] for w in wk4]
            w_l4 = [w[:, 384:480] for w in wk4]
            w_pm = [w[:, 0:128] for w in wk3]
            w_pl = [w[:, 128:150] for w in wk3]
            w_a2m = [w[:, 0:128] for w in wk1]
            w_a2l = [w[:, 128:150] for w in wk1]
            w_s2m = [w[:, 150:278] for w in wk1]
            w_s2l = [w[:, 278:300] for w in wk1]
            w_a3 = [w[:, 300:301] for w in wk1]
            w_s3 = [w[:, 301:302] for w in wk1]

            idx2_sb = wp.tile([128, 2 * (SMAX // 16)], I16, name="idx2_sb")
            idxs_sb = idx2_sb[:, 0:SMAX // 16]
            idxe_sb = idx2_sb[:, SMAX // 16:2 * (SMAX // 16)]
            mhs_sb = wp.tile([5, SMAX], BF16, name="mhs_sb")

            ident = wp.tile([128, 128], F32, name="ident")
            make_identity(nc, ident[:])
            identb = wp.tile([128, 128], BF16, name="identb")
            make_identity(nc, identb[:])
            # row-0 C-part of each stage table must be finite: real data for
            # stage 0 (exclusive prefix of token 0 == 0) and padded span slots
            # in every stage gather row 0
            zrow = wp.tile([1, 302], BF16, name="zrow")
            nc.vector.memset(zrow[:], 0.0)
            for tb in tabs:
                nc.scalar.dma_start(tb[0:1, 150:452], zrow[:])

            # ================= span phase =================
            scstate = {"scs": None}

            def span_group(c0, g1, g2, kk0, ng):
                """One group of ng (4 or 2) span tiles, stage-batched.
                g1 window = [A | C | .]; g2 window = [C | B]."""
                W = ng * 128
                # psw[i] = A[start] + B[end] + width-term  (PE, PSUM accum).
                # identity adds first so the scheduler cannot hoist the
                # mh-matmul ahead of the gathers; 1024B stride keeps each
                # output inside one bank
                pswA = ps.tile([128, 2, 256], F32, name="pswA", tag="psw", bufs=2)
                pswv = [pswA[:, 0, 0:150], pswA[:, 1, 0:150]]
                if ng >= 3:
                    pswB = ps.tile([128, 2, 256], F32, name="pswB", tag="psw", bufs=2)
                    pswv += [pswB[:, 0, 0:150], pswB[:, 1, 0:150]]
                for i in range(ng):
                    nc.tensor.matmul(pswv[i], lhsT=identb[:], rhs=g1[:, kk0 + i, 0:150],
                                     start=True, stop=False)
                    nc.tensor.matmul(pswv[i], lhsT=identb[:], rhs=g2[:, kk0 + i, 302:452],
                                     start=False, stop=False)
                    nc.tensor.matmul(pswv[i], lhsT=mhs_sb[:, (c0 + i) * 128:(c0 + i + 1) * 128],
                                     rhs=w_dt, start=False, stop=True)

                # grouped prefix difference: [128, 4, 151] f32
                c0v = g1[:, kk0:kk0 + ng, 150:452].bitcast(F32)
                c1v = g2[:, kk0:kk0 + ng, 0:302].bitcast(F32)
                diffg = sp4.tile([128, ng, 151], F32, name="diffg", tag="diffg")
                nc.vector.tensor_sub(diffg[:], c1v, c0v)
                rec = sp4.tile([128, ng, 1], F32, name="rec", tag="rec")
                nc.vector.reciprocal(rec[:], diffg[:, :, 150:151])

                # h1 = relu(diff*rec + psw): fused multiply-add, then clamp
                h1p = sp4.tile([128, ng, 150], F32, name="h1p", tag="h1p")
                for i in range(ng):
                    nc.vector.scalar_tensor_tensor(
                        h1p[:, i, :], diffg[:, i, 0:150], rec[:, i, 0:1],
                        pswv[i], op0=OP.mult, op1=OP.add)
                h1s = sp4.tile([128, ng, 150], BF16, name="h1s", tag="h1s")
                nc.vector.tensor_scalar(h1s[:], h1p[:], 0.0, None, op0=OP.max)

                # transpose all 4 tiles into one [150, 512] pair packed in a
                # single PSUM bank ([128, 1024] bf16: main at 0:512, tail at
                # 512:1024 on partitions 0:22)
                pT = ps.tile([128, 1024], BF16, name="pT", tag="psp", bufs=1)
                for i in range(ng):
                    nc.tensor.transpose(pT[:, i * 128:(i + 1) * 128],
                                        h1s[:, i, 0:128], identb[:])
                    nc.tensor.transpose(pT[0:22, 512 + i * 128:512 + (i + 1) * 128],
                                        h1s[:, i, 128:150], identb[:])
                h1t = sp3.tile([128, 1024], BF16, name="h1t", tag="h1t")
                nc.any.tensor_copy(h1t[:, 0:W], pT[:, 0:W])
                nc.any.tensor_copy(h1t[0:22, 512:512 + W], pT[0:22, 512:512 + W])
                h1tg = h1t[:, 0:W]
                h1tl = h1t[0:22, 512:512 + W]

                # layer 2 + ReLU, 512 wide; layer-3 output rides row 32 of the
                # tail bank so no extra PSUM bank is needed
                pH2g = ps.tile([128, W], F32, name="pH2g", tag="psp", bufs=1)
                nc.tensor.matmul(pH2g[:], lhsT=w_s2m[0], rhs=h1tg, start=True, stop=False)
                nc.tensor.matmul(pH2g[:], lhsT=w_s2m[1], rhs=h1tl, start=False, stop=True)
                pH2x = ps.tile([33, W], F32, name="pH2x", tag="pH2x", bufs=1)
                nc.tensor.matmul(pH2x[0:22, :], lhsT=w_s2l[0], rhs=h1tg, start=True, stop=False)
                nc.tensor.matmul(pH2x[0:22, :], lhsT=w_s2l[1], rhs=h1tl, start=False, stop=True)
                h2g = sp3.tile([128, W], BF16, name="h2g", tag="h2g")
                nc.scalar.activation(h2g[:], pH2g[:], AF.Relu, bias=b1[0:128, 4:5])
                h2l = sp3.tile([22, W], BF16, name="h2l", tag="h2l")
                nc.scalar.activation(h2l[:], pH2x[0:22, :], AF.Relu, bias=b1[0:22, 5:6])

                # layer 3 -> scores
                nc.tensor.matmul(pH2x[32:33, :], lhsT=w_s3[0], rhs=h2g[:], start=True, stop=False)
                nc.tensor.matmul(pH2x[32:33, :], lhsT=w_s3[1], rhs=h2l[:], start=False, stop=True)

                scs = sp2.tile([1, W], F32, name="scs", tag="scs")
                nc.vector.tensor_scalar(scs[:], pH2x[32:33, :],
                                        b1[0:1, 6:7], None, op0=OP.add)
                nc.sync.dma_start(scoresd[0:1, c0 * 128:c0 * 128 + W], scs[:])

            def emit_chunk(c0, c1, tb):
                """Gather c1-c0 span tiles, compute them in 512-span groups."""
                nt = c1 - c0
                g1 = gp.tile([128, nt, 512], BF16, name=f"g1_{nt}",
                             tag=f"g1_{nt}", bufs=2 if nt == 8 else 1)
                nc.gpsimd.dma_gather(
                    out_ap=g1[:], in_ap=tb[:, 0:512],
                    idxs_ap=idxs_sb[:, c0 * 8:c1 * 8],
                    num_idxs=nt * 128, num_idxs_reg=nt * 128, elem_size=512,
                    elem_step=1024)
                g2 = gp.tile([128, nt, 512], BF16, name=f"g2_{nt}",
                             tag=f"g2_{nt}", bufs=2 if nt == 8 else 1)
                nc.gpsimd.dma_gather(
                    out_ap=g2[:], in_ap=tb[:, 150:662],
                    idxs_ap=idxe_sb[:, c0 * 8:c1 * 8],
                    num_idxs=nt * 128, num_idxs_reg=nt * 128, elem_size=512,
                    elem_step=1024)
                k = 0
                while k < nt:
                    g = 4 if nt - k >= 4 else nt - k
                    span_group(c0 + k, g1, g2, k, g)
                    k += g

            # ================= token phase =================
            # software pipeline: compute(b) then export(b-1); export does the
            # table transposes/copies/DMA writes.
            def export_block(bi, t0, TB, A1, B1, packed, C1, C2):
                stage = 0 if t0 + TB <= SPLIT1 else (1 if t0 + TB <= SPLIT2 else 2)
                w1 = tabs[stage]
                r0 = t0 - (0, OFF2, OFF3)[stage]
                nj = TB // 128
                afull = tok.tile([128, nj, 150], BF16, name="afull", tag="afull")
                bfull = tok.tile([128, nj, 150], BF16, name="bfull", tag="bfull")
                cfull = tok.tile([128, nj, 151], F32, name="cfull", tag="cfull")
                for j in range(nj):
                    js = j * 128
                    psE = ps.tile([128, 512], F32, name="psE", tag="pec", bufs=2)
                    psAB = psE[:, 0:150].bitcast(BF16)
                    psC = psE[:, 256:407]
                    nc.tensor.transpose(psAB[:, 0:128], A1[:, js:js + 128], identb[:])
                    nc.tensor.transpose(psAB[:, 128:150], packed[0:22, js:js + 128],
                                        identb[0:22, 0:22])
                    nc.tensor.transpose(psAB[:, 150:278], B1[:, js:js + 128], identb[:])
                    nc.tensor.transpose(psAB[:, 278:300], packed[32:54, js:js + 128],
                                        identb[32:54, 32:54])
                    nc.tensor.transpose(psC[:, 0:128], C1[:, js:js + 128], ident[:])
                    nc.tensor.transpose(psC[:, 128:151], C2[0:23, js:js + 128],
                                        ident[0:23, 0:23])
                    nc.any.tensor_copy(afull[:, j, :], psAB[:, 0:150])
                    nc.any.tensor_copy(bfull[:, j, :], psAB[:, 150:300])
                    nc.any.tensor_copy(cfull[:, j, :], psC[:])

                nc.scalar.dma_start(
                    w1[r0:r0 + TB, 0:150].rearrange("(j p) c -> p j c", p=128),
                    afull[:])
                nc.sync.dma_start(
                    w1[r0 + 1:r0 + TB + 1, 452:602].rearrange("(j p) c -> p j c", p=128),
                    bfull[:])
                nc.scalar.dma_start(
                    w1[r0 + 1:r0 + TB + 1, 150:452].bitcast(F32)
                    .rearrange("(j p) c -> p j c", p=128),
                    cfull[:])
                if t0 + TB in (SPLIT1, SPLIT2):
                    # 32-row sliver into the next stage's table: spans whose
                    # end sits just past the boundary reach back <= 10 rows
                    nw = tabs[stage + 1]
                    nc.sync.dma_start(nw[0:32, 0:150], afull[96:128, 3, :])
                    nc.scalar.dma_start(nw[1:33, 452:602], bfull[96:128, 3, :])
                    nc.sync.dma_start(nw[1:33, 150:452].bitcast(F32),
                                      cfull[96:128, 3, :])

            prevC1 = prevC2 = None
            prevTB = 0
            pending = None
            # chunk firing schedule: stage-s chunks become legal one block
            # after the export covering that stage's rows is emitted
            ends = [t0 + TB for (t0, TB) in TOK_BLOCKS]
            legal1 = ends.index(SPLIT1) + 3
            legal2 = ends.index(SPLIT2) + 2
            chunk_list = []   # (c0, c1, stage, legal_bi)
            base = 0
            for s, nts in enumerate((NT1, NT2, NT3)):
                legal = (legal1, legal2, len(TOK_BLOCKS))[s]
                for c0 in range(base, base + nts - 7, 8):
                    chunk_list.append((c0, c0 + 8, s, legal))
                rem = nts % 8
                if rem:
                    chunk_list.append((base + nts - rem, base + nts, s, legal))
                base += nts
            fire_at = {legal1 + i: 1 for i in range(legal2 - legal1)}
            fire_at.update({legal2 + i: 2 for i in range(len(TOK_BLOCKS) - legal2)})
            next_g = [0]

            def fire_chunks(bi, budget):
                while (budget > 0 and next_g[0] < len(chunk_list)
                       and chunk_list[next_g[0]][3] <= bi):
                    c0, c1, s, _ = chunk_list[next_g[0]]
                    emit_chunk(c0, c1, tabs[s])
                    next_g[0] += 1
                    budget -= 1

            for bi, (t0, TB) in enumerate(TOK_BLOCKS):
                seb = tok.tile([128, 7, TB], BF16, name="seb", tag="seb")
                nc.sync.dma_start(
                    seb[:], seTd[:, t0:t0 + TB].rearrange("(j p) t -> p j t", p=128))
                st = [seb[0:128, 0, :], seb[0:128, 1, :], seb[0:128, 2, :],
                      seb[0:16, 3, :]]
                et = [seb[0:128, 4, :], seb[0:128, 5, :], seb[0:94, 6, :]]

                def mm_group(shape, lhs_list, rhs_list, name, tag="pst"):
                    p = ps.tile(shape, F32, name=name, tag=tag,
                                bufs=2)
                    n = len(lhs_list)
                    for i in range(n):
                        nc.tensor.matmul(p[:], lhsT=lhs_list[i], rhs=rhs_list[i],
                                         start=(i == 0), stop=(i == n - 1))
                    return p

                # each PSUM mm output is consumed immediately after it is
                # produced so ring slots turn over fast (short PSUM lifetimes)
                pH1 = mm_group([128, TB], w_aw1, st, "pH1")
                h1a = tok.tile([128, TB], BF16, name="h1a", tag="h1a")
                nc.scalar.activation(h1a[:], pH1[:], AF.Relu, bias=b1[0:128, 0:1])

                pA = mm_group([128, TB], w_sa, st, "pA")
                A1 = tok.tile([128, TB], BF16, name="A1", tag="A1")
                nc.any.tensor_copy(A1[:], pA[:])
                pB = mm_group([128, TB], w_sb, st, "pB")
                B1 = tok.tile([128, TB], BF16, name="B1", tag="B1")
                nc.any.tensor_copy(B1[:], pB[:])

                pL = mm_group([96, TB], w_l4, st, "pL")
                h1b = tok.tile([22, TB], BF16, name="h1b", tag="h1b")
                nc.scalar.activation(h1b[:], pL[0:22, :], AF.Relu, bias=b1[0:22, 1:2])
                packed = tok.tile([54, TB], BF16, name="packed", tag="packed")
                nc.any.tensor_copy(packed[0:22, :], pL[32:54, :])
                nc.any.tensor_copy(packed[32:54, :], pL[64:86, :])

                pH2 = mm_group([128, TB], w_a2m, [h1a[:], h1b[:]], "pH2")
                pH2l = mm_group([22, TB], w_a2l, [h1a[:], h1b[:]], "pH2l")
                h2a = tok.tile([128, TB], BF16, name="h2a", tag="h2a")
                nc.scalar.activation(h2a[:], pH2[:], AF.Relu, bias=b1[0:128, 2:3])
                h2b = tok.tile([22, TB], BF16, name="h2b", tag="h2b")
                nc.scalar.activation(h2b[:], pH2l[:], AF.Relu, bias=b1[0:22, 3:4])

                pAt = mm_group([1, TB], w_a3, [h2a[:], h2b[:]], "pAt")
                e_sb = tok.tile([1, TB], F32, name="e_sb", tag="e_sb")
                nc.scalar.activation(e_sb[:], pAt[0:1, :], AF.Exp, bias=b1[0:1, 7:8])

                # broadcast e across partitions on the Pool engine
                ebc = tok.tile([128, TB], F32, name="ebc", tag="ebc")
                nc.gpsimd.partition_broadcast(ebc[:], e_sb[0:1, :])

                # EC matmuls emitted here (not at block top) so their PSUM
                # slots are held only briefly before EV1/EV2 consume them
                pEC = mm_group([128, TB], w_pm, et, "pEC", tag="pec")
                pECl = mm_group([22, TB], w_pl, et, "pECl", tag="pec")

                EV1 = tok.tile([128, TB], F32, name="EV1", tag="EV1")
                nc.vector.tensor_mul(EV1[:], pEC[:], ebc[:])
                # e rides row 22 via DMA (engine ops would need a 32-aligned
                # partition offset; DMA writes have no such restriction)
                EV2 = tok.tile([23, TB], F32, name="EV2", tag="EV2")
                nc.vector.tensor_mul(EV2[0:22, :], pECl[:], ebc[0:22, :])
                nc.scalar.dma_start(EV2[22:23, :], ebc[0:1, :])

                C1 = tok.tile([128, TB], F32, name="C1", tag="c1")
                init1 = 0.0 if prevC1 is None else prevC1[:, prevTB - 1:prevTB]
                nc.vector.tensor_tensor_scan(C1[:], EV1[:], EV1[:], init1,
                                             op0=OP.add, op1=OP.bypass)
                C2 = tok.tile([23, TB], F32, name="C2", tag="c2")
                init2 = 0.0 if prevC2 is None else prevC2[:, prevTB - 1:prevTB]
                nc.vector.tensor_tensor_scan(C2[:], EV2[:], EV2[:], init2,
                                             op0=OP.add, op1=OP.bypass)
                prevC1, prevC2, prevTB = C1, C2, TB

                if pending is not None:
                    export_block(*pending)
                pending = (bi, t0, TB, A1, B1, packed, C1, C2)

                fire_chunks(bi, fire_at.get(bi, 0))
                if bi == 0:
                    # span-phase constants aren't needed until bi>=5; loading
                    # them here keeps the startup DMA queue clear
                    nc.sync.dma_start(idx2_sb[:], idxd[:, :])
                    nc.scalar.dma_start(mhs_sb[:], mhd[:, :])

            export_block(*pending)
            fire_chunks(len(TOK_BLOCKS), len(chunk_list))

    nc.compile()
    return nc


def _prep_shared(inputs):
    """Host-side weight packing into one [128, WTOT] bf16 wall."""
    f32 = lambda x: np.ascontiguousarray(np.asarray(x), dtype=np.float32)
    aw1, ab1 = f32(inputs["aw1"]), f32(inputs["ab1"])
    aw2, ab2 = f32(inputs["aw2"]), f32(inputs["ab2"])
    aw3, ab3 = f32(inputs["aw3"]), f32(inputs["ab3"])
    sw1, sb1 = f32(inputs["sw1"]), f32(inputs["sb1"])
    sw2, sb2 = f32(inputs["sw2"]), f32(inputs["sb2"])
    sw3, sb3 = f32(inputs["sw3"]), f32(inputs["sb3"])
    wt = f32(inputs["width_table"])

    sw1a, sw1b, sw1p, sw1w = sw1[0:400], sw1[400:800], sw1[800:1150], sw1[1150:1170]

    wl = np.zeros((DS, 96), np.float32)
    wl[:, 0:22] = aw1[:, 128:150]
    wl[:, 32:54] = sw1a[:, 128:150]
    wl[:, 64:86] = sw1b[:, 128:150]

    wall = np.zeros((128, WTOT), BF16NP)
    wk4 = np.concatenate([aw1[:, 0:128], sw1a[:, 0:128], sw1b[:, 0:128], wl], axis=1)
    for i, (k0, k1) in enumerate(K400):
        wall[0:k1 - k0, WOFF[i]:WOFF[i] + 480] = wk4[k0:k1].astype(BF16NP)
    for i, (k0, k1) in enumerate(K350):
        wall[0:k1 - k0, WOFF[4 + i]:WOFF[4 + i] + 150] = sw1p[k0:k1].astype(BF16NP)
    wk1 = np.concatenate([aw2, sw2, aw3, sw3], axis=1)
    for i, (k0, k1) in enumerate(K150):
        wall[0:k1 - k0, WOFF[7 + i]:WOFF[7 + i] + 302] = wk1[k0:k1].astype(BF16NP)

    # width-bin difference table with sb1 folded in (widths are 1..10 -> bin 1..5)
    Wmb = wt @ sw1w  # [9, 150]
    dtab = np.zeros((5, H), np.float32)
    dtab[0] = Wmb[1] + sb1
    for jj in range(1, 5):
        dtab[jj] = Wmb[jj + 1] - Wmb[jj]
    wall[0:5, WOFF[9]:WOFF[9] + 150] = dtab.astype(BF16NP)

    b1p = np.zeros((128, 8), np.float32)
    b1p[:, 0] = ab1[0:128]
    b1p[0:22, 1] = ab1[128:150]
    b1p[:, 2] = ab2[0:128]
    b1p[0:22, 3] = ab2[128:150]
    b1p[:, 4] = sb2[0:128]
    b1p[0:22, 5] = sb2[128:150]
    b1p[0, 6] = sb3[0]
    b1p[0, 7] = ab3[0]
    wall[:, WOFF[10]:WOFF[10] + 16] = b1p.view(BF16NP)
    return wall


def _wrap16(a):
    """int16 index array in the SWDGE layout: idx i at [i%16, i//16],
    replicated 8x across the 128 partitions (one copy per Q7 core)."""
    w = np.ascontiguousarray(a.astype(np.int16).reshape(-1, 16).T)
    return np.ascontiguousarray(np.tile(w, (8, 1)))


def prepare_in_maps(inputs):
    """Host-side sharding: returns (in_maps, sels) — per-core input dicts and
    the original span indices each core's padded slots map back to."""
    states = np.asarray(inputs["states"], dtype=np.float32)
    embeds = np.asarray(inputs["embeds"], dtype=np.float32)
    starts = np.asarray(inputs["span_starts"]).astype(np.int64)
    widths = np.asarray(inputs["span_widths"]).astype(np.int64)

    wall = _prep_shared(inputs)

    bucket = np.minimum(starts // TPC, NCORES - 1)
    order = np.argsort(bucket, kind="stable")
    counts = np.bincount(bucket, minlength=NCORES)
    assert counts.max() <= SMAX, f"span bucket overflow: {counts.max()} > {SMAX}"
    offs = np.zeros(NCORES + 1, np.int64)
    offs[1:] = np.cumsum(counts)

    mh_full = (widths[None, :] >= BINS5[:, None]).astype(np.float32)  # [5, S]

    stage_caps = [NT1 * 128, NT2 * 128, NT3 * 128]
    stage_offs = [0, OFF2, OFF3]
    slot0 = [0, NT1 * 128, (NT1 + NT2) * 128]

    in_maps = []
    sels = []
    for cix in range(NCORES):
        t0 = cix * TPC
        tl = min(T, t0 + TPC + W_MAX - 1) - t0
        seT = np.zeros((896, TL_PAD), BF16NP)
        seT[0:DS, :tl] = states[t0:t0 + tl].T.astype(BF16NP)
        seT[512:512 + DE, :tl] = embeds[t0:t0 + tl].T.astype(BF16NP)

        sel = order[offs[cix]:offs[cix + 1]]
        lsr = (starts[sel] - t0).astype(np.int32)
        ler = lsr + widths[sel].astype(np.int32) - 1

        slot_orig = np.full(SMAX, -1, np.int64)
        ls = np.zeros(SMAX, np.int32)
        le = np.zeros(SMAX, np.int32)
        mh = np.zeros((5, SMAX), BF16NP)
        stg = np.where(ler < SPLIT1, 0, np.where(ler < SPLIT2, 1, 2))
        for s in range(3):
            ix = np.nonzero(stg == s)[0]
            assert len(ix) <= stage_caps[s], f"stage {s} overflow: {len(ix)}"
            base, off = slot0[s], stage_offs[s]
            slot_orig[base:base + len(ix)] = sel[ix]
            # local indices for this stage's table; pad slots keep
            # ls=0 / le=0 (-> gather rows 0 and 1, both finite)
            ls[base:base + len(ix)] = lsr[ix] - off
            le[base:base + len(ix)] = ler[ix] - off
            mh[:, base:base + len(ix)] = mh_full[:, sel[ix]].astype(BF16NP)
        sels.append(slot_orig)

        in_maps.append({
            "seT": seT,
            "idx2": np.concatenate([_wrap16(ls), _wrap16(le + 1)], axis=1),
            "mh": mh,
            "wall": wall,
        })
    return in_maps, sels


def kernel(**inputs) -> np.ndarray:
    in_maps, sels = prepare_in_maps(inputs)

    if "nc" not in _PROGRAM_CACHE:
        _PROGRAM_CACHE["nc"] = _build_program()
    nc = _PROGRAM_CACHE["nc"]

    from concourse.bass_utils import run_bass_kernel_spmd
    res = run_bass_kernel_spmd(nc, in_maps, core_ids=list(range(NCORES)))
    _PROGRAM_CACHE["last_res"] = res  # exec_time_ns etc, for the test harness

    out = np.zeros(S, np.float32)
    for cix in range(NCORES):
        slot_orig = sels[cix]
        m = slot_orig >= 0
        vals = np.asarray(res.results[cix]["scores"]).reshape(-1)
        out[slot_orig[m]] = vals[m]
    return out


# revision 84
# speedup vs baseline: 1.0090x; 1.0090x over previous
"""MentionScore fused Bass kernel for 8 Trainium2 NeuronCores.

Strategy (self-contained, hardcoded for the nn_MentionScore problem):
  - Spans are bucketed by start//6250 -> one bucket per core; each core only
    needs its 6250-token slice (+9 halo), so states/embeds are sharded with
    no collectives.
  - Token phase (feature-major): per-token attention-logit MLP -> e = exp(a);
    the span MLP's first layer is decomposed through the gathers:
        g @ sw1 = A[start] + B[end] + (pooled @ sw1_p) + width-term
    with A = states@sw1[0:400], B = states@sw1[400:800], and
    pooled @ sw1_p = (P[end] - P[start-1]) / (Pe[end] - Pe[start-1]) where
    [P | Pe] = inclusive prefix sums of [e*EC | e], EC = embeds@sw1[800:1150].
    Prefix sums run on the vector engine (tensor_tensor_scan along tokens).
    The token loop is software-pipelined: block b's table export (PE
    transposes + PSUM->SBUF copies + table DMAs) is emitted after block
    b+1's compute stage so the PE queue never head-blocks on the scans.
  - Tables are token-major with 2048B-stride rows laid out
        tabX[t] = [A[t] (150 bf16) | C[t-1] (151 f32) | B[t-1] (150 bf16)]
    so one batched SWDGE dma_gather window [0:512) at `start` yields
    A[start] + the exclusive prefix, and a second window [150:662) at
    `end+1` yields the inclusive prefix + B[end] — C is written once and
    shared by both windows (994ns SWDGE fixed cost paid per chunk instead
    of per 128-span tile; elem_step=1024 strides the 512-elem windows).
  - Tables come in three stages (token ranges [0,2560), [2560,5120), rest).
    Spans are bucketed by `end`; since width <= 10 both gather indices of a
    span live in its stage's range (+small sliver), so each table row is
    written exactly once and span chunks fire as soon as their stage's
    token blocks are exported.
  - Span phase is emitted stage-batched over groups of 4 tiles (512 spans):
    per-tile ops are fused (scalar_tensor_tensor joins the pooled-term
    normalize with the PSUM accumulator) and layer-2/3 matmuls + ReLUs run
    512 wide, so each engine queue streams instead of ping-ponging.
"""

import ml_dtypes
import numpy as np

BF16NP = ml_dtypes.bfloat16

# ---- problem constants (hardcoded per contract) ----
T, S = 50000, 100000
DS, DE, H, DW = 400, 350, 150, 20
W_MAX = 10
BINS5 = np.array([1, 2, 3, 4, 8], np.int64)
NCORES = 8
TPC = T // NCORES            # 6250 tokens per core bucket
TL_PAD = 6272                # 49 * 128 padded local tokens (6250 + 9 halo -> 6259)
NT_S = 102                   # span tiles per core
SMAX = NT_S * 128            # 14336 padded spans per core
TOK_BLOCKS = [(0, 128), (128, 384)] + [(i * 512, 512) for i in range(1, 12)] + [(6144, 128)]
# span stage buckets by end-token: [0,2560) / [2560,5120) / [5120,...)
SPLIT1, SPLIT2 = 2560, 5120
NT1, NT2, NT3 = 42, 41, 19   # span tiles per stage (max observed 5253/5174/2373)
OFF2, OFF3 = 2528, 5088      # table row offsets (32-aligned slivers)
ROWS1, ROWS2, ROWS3 = 2576, 2624, 1312
K400 = [(0, 128), (128, 256), (256, 384), (384, 400)]
K350 = [(0, 128), (128, 256), (256, 350)]
K150 = [(0, 128), (128, 150)]
# packed weight wall: [wk4 x4 | wk3 x3 | wk1 x2 | dtab | bias(16 bf16-cols)]
WCOLS = [480] * 4 + [150] * 3 + [302] * 2 + [150] + [16]
WOFF = np.concatenate([[0], np.cumsum(WCOLS)]).astype(int)
WTOT = int(WOFF[-1])

_PROGRAM_CACHE = {}


def _build_program():
    import concourse.bacc as bacc
    import concourse.bass as bass
    import concourse.mybir as mybir
    import concourse.tile as tile
    from concourse.masks import make_identity

    F32 = mybir.dt.float32
    BF16 = mybir.dt.bfloat16
    I16 = mybir.dt.int16
    AF = mybir.ActivationFunctionType
    OP = mybir.AluOpType

    nc = bacc.Bacc("TRN2", num_devices=NCORES)

    # ---- I/O ----
    # packed [states.T (400, pad to 512) | embeds.T (350, pad to 384)] rows
    seTd = nc.dram_tensor("seT", [896, TL_PAD], BF16, kind="ExternalInput")
    walld = nc.dram_tensor("wall", [128, WTOT], BF16, kind="ExternalInput")
    idxd = nc.dram_tensor("idx2", [128, 2 * (SMAX // 16)], I16, kind="ExternalInput")
    mhd = nc.dram_tensor("mh", [5, SMAX], BF16, kind="ExternalInput")
    scoresd = nc.dram_tensor("scores", [1, SMAX], F32, kind="ExternalOutput")

    tabs = [nc.dram_tensor(f"tabX_{i}", [rows, 1024], BF16)
            for i, rows in enumerate([ROWS1, ROWS2, ROWS3])]

    with tile.TileContext(nc) as tc:
        with (
            tc.tile_pool(name="wpool", bufs=1) as wp,
            tc.tile_pool(name="tok", bufs=3) as tok,
            tc.tile_pool(name="sp4", bufs=3) as sp4,
            tc.tile_pool(name="sp2", bufs=2) as sp2,
            tc.tile_pool(name="sp3", bufs=3) as sp3,
            tc.tile_pool(name="gat", bufs=2) as gp,
            tc.tile_pool(name="ps", bufs=1, space="PSUM") as ps,
        ):
            # ---- resident weights / constants (one DMA for the wall) ----
            wall = wp.tile([128, WTOT], BF16, name="wall")
            nc.sync.dma_start(wall[:], walld[:, :])
            wk4 = [wall[0:k1 - k0, WOFF[i]:WOFF[i] + 480] for i, (k0, k1) in enumerate(K400)]
            wk3 = [wall[0:k1 - k0, WOFF[4 + i]:WOFF[4 + i] + 150] for i, (k0, k1) in enumerate(K350)]
            wk1 = [wall[0:k1 - k0, WOFF[7 + i]:WOFF[7 + i] + 302] for i, (k0, k1) in enumerate(K150)]
            w_dt = wall[0:5, WOFF[9]:WOFF[9] + 150]
            b1 = wall[:, WOFF[10]:WOFF[10] + 16].bitcast(F32)

            w_aw1 = [w[:, 0:128] for w in wk4]
            w_sa = [w[:, 128:256] for w in wk4]
            w_sb = [w[:, 256:384] for w in wk4]
            w_l4 = [w[:, 384:480] for w in wk4]
            w_pm = [w[:, 0:128] for w in wk3]
            w_pl = [w[:, 128:150] for w in wk3]
            w_a2m = [w[:, 0:128] for w in wk1]
            w_a2l = [w[:, 128:150] for w in wk1]
            w_s2m = [w[:, 150:278] for w in wk1]
            w_s2l = [w[:, 278:300] for w in wk1]
            w_a3 = [w[:, 300:301] for w in wk1]
            w_s3 = [w[:, 301:302] for w in wk1]

            idx2_sb = wp.tile([128, 2 * (SMAX // 16)], I16, name="idx2_sb")
            idxs_sb = idx2_sb[:, 0:SMAX // 16]
            idxe_sb = idx2_sb[:, SMAX // 16:2 * (SMAX // 16)]
            mhs_sb = wp.tile([5, SMAX], BF16, name="mhs_sb")

            ident = wp.tile([128, 128], F32, name="ident")
            make_identity(nc, ident[:])
            identb = wp.tile([128, 128], BF16, name="identb")
            make_identity(nc, identb[:])
            # row-0 C-part of each stage table must be finite: real data for
            # stage 0 (exclusive prefix of token 0 == 0) and padded span slots
            # in every stage gather row 0
            zrow = wp.tile([1, 302], BF16, name="zrow")
            nc.vector.memset(zrow[:], 0.0)
            for tb in tabs:
                nc.scalar.dma_start(tb[0:1, 150:452], zrow[:])

            # ================= span phase =================
            scstate = {"scs": None}

            def span_group(c0, g1, g2, kk0, ng):
                """One group of ng (4 or 2) span tiles, stage-batched.
                g1 window = [A | C | .]; g2 window = [C | B]."""
                W = ng * 128
                # psw[i] = A[start] + B[end] + width-term  (PE, PSUM accum).
                # identity adds first so the scheduler cannot hoist the
                # mh-matmul ahead of the gathers; 1024B stride keeps each
                # output inside one bank
                pswA = ps.tile([128, 2, 256], F32, name="pswA", tag="psw", bufs=2)
                pswv = [pswA[:, 0, 0:150], pswA[:, 1, 0:150]]
                if ng >= 3:
                    pswB = ps.tile([128, 2, 256], F32, name="pswB", tag="psw", bufs=2)
                    pswv += [pswB[:, 0, 0:150], pswB[:, 1, 0:150]]
                for i in range(ng):
                    nc.tensor.matmul(pswv[i], lhsT=identb[:], rhs=g1[:, kk0 + i, 0:150],
                                     start=True, stop=False)
                    nc.tensor.matmul(pswv[i], lhsT=identb[:], rhs=g2[:, kk0 + i, 302:452],
                                     start=False, stop=False)
                    nc.tensor.matmul(pswv[i], lhsT=mhs_sb[:, (c0 + i) * 128:(c0 + i + 1) * 128],
                                     rhs=w_dt, start=False, stop=True)

                # grouped prefix difference: [128, 4, 151] f32
                c0v = g1[:, kk0:kk0 + ng, 150:452].bitcast(F32)
                c1v = g2[:, kk0:kk0 + ng, 0:302].bitcast(F32)
                diffg = sp4.tile([128, ng, 151], F32, name="diffg", tag="diffg")
                nc.vector.tensor_sub(diffg[:], c1v, c0v)
                rec = sp4.tile([128, ng, 1], F32, name="rec", tag="rec")
                nc.vector.reciprocal(rec[:], diffg[:, :, 150:151])

                # h1 = relu(diff*rec + psw): fused multiply-add, then clamp
                h1p = sp4.tile([128, ng, 150], F32, name="h1p", tag="h1p")
                for i in range(ng):
                    nc.vector.scalar_tensor_tensor(
                        h1p[:, i, :], diffg[:, i, 0:150], rec[:, i, 0:1],
                        pswv[i], op0=OP.mult, op1=OP.add)
                h1s = sp4.tile([128, ng, 150], BF16, name="h1s", tag="h1s")
                nc.vector.tensor_scalar(h1s[:], h1p[:], 0.0, None, op0=OP.max)

                # transpose all 4 tiles into one [150, 512] pair packed in a
                # single PSUM bank ([128, 1024] bf16: main at 0:512, tail at
                # 512:1024 on partitions 0:22)
                pT = ps.tile([128, 1024], BF16, name="pT", tag="psp", bufs=1)
                for i in range(ng):
                    nc.tensor.transpose(pT[:, i * 128:(i + 1) * 128],
                                        h1s[:, i, 0:128], identb[:])
                    nc.tensor.transpose(pT[0:22, 512 + i * 128:512 + (i + 1) * 128],
                                        h1s[:, i, 128:150], identb[:])
                h1t = sp3.tile([128, 1024], BF16, name="h1t", tag="h1t")
                nc.any.tensor_copy(h1t[:, 0:W], pT[:, 0:W])
                nc.any.tensor_copy(h1t[0:22, 512:512 + W], pT[0:22, 512:512 + W])
                h1tg = h1t[:, 0:W]
                h1tl = h1t[0:22, 512:512 + W]

                # layer 2 + ReLU, 512 wide; layer-3 output rides row 32 of the
                # tail bank so no extra PSUM bank is needed
                pH2g = ps.tile([128, W], F32, name="pH2g", tag="psp", bufs=1)
                nc.tensor.matmul(pH2g[:], lhsT=w_s2m[0], rhs=h1tg, start=True, stop=False)
                nc.tensor.matmul(pH2g[:], lhsT=w_s2m[1], rhs=h1tl, start=False, stop=True)
                pH2x = ps.tile([33, W], F32, name="pH2x", tag="pH2x", bufs=1)
                nc.tensor.matmul(pH2x[0:22, :], lhsT=w_s2l[0], rhs=h1tg, start=True, stop=False)
                nc.tensor.matmul(pH2x[0:22, :], lhsT=w_s2l[1], rhs=h1tl, start=False, stop=True)
                h2g = sp3.tile([128, W], BF16, name="h2g", tag="h2g")
                nc.scalar.activation(h2g[:], pH2g[:], AF.Relu, bias=b1[0:128, 4:5])
                h2l = sp3.tile([22, W], BF16, name="h2l", tag="h2l")
                nc.scalar.activation(h2l[:], pH2x[0:22, :], AF.Relu, bias=b1[0:22, 5:6])

                # layer 3 -> scores
                nc.tensor.matmul(pH2x[32:33, :], lhsT=w_s3[0], rhs=h2g[:], start=True, stop=False)
                nc.tensor.matmul(pH2x[32:33, :], lhsT=w_s3[1], rhs=h2l[:], start=False, stop=True)

                scs = sp2.tile([1, W], F32, name="scs", tag="scs")
                nc.vector.tensor_scalar(scs[:], pH2x[32:33, :],
                                        b1[0:1, 6:7], None, op0=OP.add)
                nc.sync.dma_start(scoresd[0:1, c0 * 128:c0 * 128 + W], scs[:])

            def emit_chunk(c0, c1, tb):
                """Gather c1-c0 span tiles, compute them in 512-span groups."""
                nt = c1 - c0
                g1 = gp.tile([128, nt, 512], BF16, name=f"g1_{nt}",
                             tag=f"g1_{nt}", bufs=2 if nt == 8 else 1)
                nc.gpsimd.dma_gather(
                    out_ap=g1[:], in_ap=tb[:, 0:512],
                    idxs_ap=idxs_sb[:, c0 * 8:c1 * 8],
                    num_idxs=nt * 128, num_idxs_reg=nt * 128, elem_size=512,
                    elem_step=1024)
                g2 = gp.tile([128, nt, 512], BF16, name=f"g2_{nt}",
                             tag=f"g2_{nt}", bufs=2 if nt == 8 else 1)
                nc.gpsimd.dma_gather(
                    out_ap=g2[:], in_ap=tb[:, 150:662],
                    idxs_ap=idxe_sb[:, c0 * 8:c1 * 8],
                    num_idxs=nt * 128, num_idxs_reg=nt * 128, elem_size=512,
                    elem_step=1024)
                k = 0
                while k < nt:
                    g = 4 if nt - k >= 4 else nt - k
                    span_group(c0 + k, g1, g2, k, g)
                    k += g

            # ================= token phase =================
            # software pipeline: compute(b) then export(b-1); export does the
            # table transposes/copies/DMA writes.
            def export_block(bi, t0, TB, A1, B1, packed, C1, C2):
                stage = 0 if t0 + TB <= SPLIT1 else (1 if t0 + TB <= SPLIT2 else 2)
                w1 = tabs[stage]
                r0 = t0 - (0, OFF2, OFF3)[stage]
                nj = TB // 128
                afull = tok.tile([128, nj, 150], BF16, name="afull", tag="afull")
                bfull = tok.tile([128, nj, 150], BF16, name="bfull", tag="bfull")
                cfull = tok.tile([128, nj, 151], F32, name="cfull", tag="cfull")
                for j in range(nj):
                    js = j * 128
                    psE = ps.tile([128, 512], F32, name="psE", tag="pec", bufs=2)
                    psAB = psE[:, 0:150].bitcast(BF16)
                    psC = psE[:, 256:407]
                    nc.tensor.transpose(psAB[:, 0:128], A1[:, js:js + 128], identb[:])
                    nc.tensor.transpose(psAB[:, 128:150], packed[0:22, js:js + 128],
                                        identb[0:22, 0:22])
                    nc.tensor.transpose(psAB[:, 150:278], B1[:, js:js + 128], identb[:])
                    nc.tensor.transpose(psAB[:, 278:300], packed[32:54, js:js + 128],
                                        identb[32:54, 32:54])
                    nc.tensor.transpose(psC[:, 0:128], C1[:, js:js + 128], ident[:])
                    nc.tensor.transpose(psC[:, 128:151], C2[0:23, js:js + 128],
                                        ident[0:23, 0:23])
                    nc.any.tensor_copy(afull[:, j, :], psAB[:, 0:150])
                    nc.any.tensor_copy(bfull[:, j, :], psAB[:, 150:300])
                    nc.any.tensor_copy(cfull[:, j, :], psC[:])

                nc.scalar.dma_start(
                    w1[r0:r0 + TB, 0:150].rearrange("(j p) c -> p j c", p=128),
                    afull[:])
                nc.sync.dma_start(
                    w1[r0 + 1:r0 + TB + 1, 452:602].rearrange("(j p) c -> p j c", p=128),
                    bfull[:])
                nc.scalar.dma_start(
                    w1[r0 + 1:r0 + TB + 1, 150:452].bitcast(F32)
                    .rearrange("(j p) c -> p j c", p=128),
                    cfull[:])
                if t0 + TB in (SPLIT1, SPLIT2):
                    # 32-row sliver into the next stage's table: spans whose
                    # end sits just past the boundary reach back <= 10 rows
                    nw = tabs[stage + 1]
                    nc.sync.dma_start(nw[0:32, 0:150], afull[96:128, 3, :])
                    nc.scalar.dma_start(nw[1:33, 452:602], bfull[96:128, 3, :])
                    nc.sync.dma_start(nw[1:33, 150:452].bitcast(F32),
                                      cfull[96:128, 3, :])

            prevC1 = prevC2 = None
            prevTB = 0
            pending = None
            # chunk firing schedule: stage-s chunks become legal one block
            # after the export covering that stage's rows is emitted
            ends = [t0 + TB for (t0, TB) in TOK_BLOCKS]
            legal1 = ends.index(SPLIT1) + 3
            legal2 = ends.index(SPLIT2) + 3
            chunk_list = []   # (c0, c1, stage, legal_bi)
            base = 0
            for s, nts in enumerate((NT1, NT2, NT3)):
                legal = (legal1, legal2, len(TOK_BLOCKS))[s]
                for c0 in range(base, base + nts - 7, 8):
                    chunk_list.append((c0, c0 + 8, s, legal))
                rem = nts % 8
                if rem:
                    chunk_list.append((base + nts - rem, base + nts, s, legal))
                base += nts
            fire_at = {legal1 + i: 1 for i in range(legal2 - legal1)}
            fire_at.update({legal2 + i: 2 for i in range(len(TOK_BLOCKS) - legal2)})
            next_g = [0]

            def fire_chunks(bi, budget):
                while (budget > 0 and next_g[0] < len(chunk_list)
                       and chunk_list[next_g[0]][3] <= bi):
                    c0, c1, s, _ = chunk_list[next_g[0]]
                    emit_chunk(c0, c1, tabs[s])
                    next_g[0] += 1
                    budget -= 1

            for bi, (t0, TB) in enumerate(TOK_BLOCKS):
                seb = tok.tile([128, 7, TB], BF16, name="seb", tag="seb")
                nc.sync.dma_start(
                    seb[:], seTd[:, t0:t0 + TB].rearrange("(j p) t -> p j t", p=128))
                st = [seb[0:128, 0, :], seb[0:128, 1, :], seb[0:128, 2, :],
                      seb[0:16, 3, :]]
                et = [seb[0:128, 4, :], seb[0:128, 5, :], seb[0:94, 6, :]]

                def mm_group(shape, lhs_list, rhs_list, name, tag="pst"):
                    p = ps.tile(shape, F32, name=name, tag=tag,
                                bufs=2)
                    n = len(lhs_list)
                    for i in range(n):
                        nc.tensor.matmul(p[:], lhsT=lhs_list[i], rhs=rhs_list[i],
                                         start=(i == 0), stop=(i == n - 1))
                    return p

                # each PSUM mm output is consumed immediately after it is
                # produced so ring slots turn over fast (short PSUM lifetimes)
                pH1 = mm_group([128, TB], w_aw1, st, "pH1")
                h1a = tok.tile([128, TB], BF16, name="h1a", tag="h1a")
                nc.scalar.activation(h1a[:], pH1[:], AF.Relu, bias=b1[0:128, 0:1])

                pA = mm_group([128, TB], w_sa, st, "pA")
                A1 = tok.tile([128, TB], BF16, name="A1", tag="A1")
                nc.any.tensor_copy(A1[:], pA[:])
                pB = mm_group([128, TB], w_sb, st, "pB")
                B1 = tok.tile([128, TB], BF16, name="B1", tag="B1")
                nc.any.tensor_copy(B1[:], pB[:])

                pL = mm_group([96, TB], w_l4, st, "pL")
                h1b = tok.tile([22, TB], BF16, name="h1b", tag="h1b")
                nc.scalar.activation(h1b[:], pL[0:22, :], AF.Relu, bias=b1[0:22, 1:2])
                packed = tok.tile([54, TB], BF16, name="packed", tag="packed")
                nc.any.tensor_copy(packed[0:22, :], pL[32:54, :])
                nc.any.tensor_copy(packed[32:54, :], pL[64:86, :])

                pH2 = mm_group([128, TB], w_a2m, [h1a[:], h1b[:]], "pH2")
                pH2l = mm_group([22, TB], w_a2l, [h1a[:], h1b[:]], "pH2l")
                h2a = tok.tile([128, TB], BF16, name="h2a", tag="h2a")
                nc.scalar.activation(h2a[:], pH2[:], AF.Relu, bias=b1[0:128, 2:3])
                h2b = tok.tile([22, TB], BF16, name="h2b", tag="h2b")
                nc.scalar.activation(h2b[:], pH2l[:], AF.Relu, bias=b1[0:22, 3:4])

                pAt = mm_group([1, TB], w_a3, [h2a[:], h2b[:]], "pAt")
                e_sb = tok.tile([1, TB], F32, name="e_sb", tag="e_sb")
                nc.scalar.activation(e_sb[:], pAt[0:1, :], AF.Exp, bias=b1[0:1, 7:8])

                # broadcast e across partitions on the Pool engine
                ebc = tok.tile([128, TB], F32, name="ebc", tag="ebc")
                nc.gpsimd.partition_broadcast(ebc[:], e_sb[0:1, :])

                # EC matmuls emitted here (not at block top) so their PSUM
                # slots are held only briefly before EV1/EV2 consume them
                pEC = mm_group([128, TB], w_pm, et, "pEC", tag="pec")
                pECl = mm_group([22, TB], w_pl, et, "pECl", tag="pec")

                EV1 = tok.tile([128, TB], F32, name="EV1", tag="EV1")
                nc.vector.tensor_mul(EV1[:], pEC[:], ebc[:])
                # e rides row 22 via DMA (engine ops would need a 32-aligned
                # partition offset; DMA writes have no such restriction)
                EV2 = tok.tile([23, TB], F32, name="EV2", tag="EV2")
                nc.vector.tensor_mul(EV2[0:22, :], pECl[:], ebc[0:22, :])
                nc.scalar.dma_start(EV2[22:23, :], ebc[0:1, :])

                C1 = tok.tile([128, TB], F32, name="C1", tag="c1")
                init1 = 0.0 if prevC1 is None else prevC1[:, prevTB - 1:prevTB]
                nc.vector.tensor_tensor_scan(C1[:], EV1[:], EV1[:], init1,
                                             op0=OP.add, op1=OP.bypass)
                C2 = tok.tile([23, TB], F32, name="C2", tag="c2")
                init2 = 0.0 if prevC2 is None else prevC2[:, prevTB - 1:prevTB]
                nc.vector.tensor_tensor_scan(C2[:], EV2[:], EV2[:], init2,
                                             op0=OP.add, op1=OP.bypass)
                prevC1, prevC2, prevTB = C1, C2, TB

                if pending is not None:
                    export_block(*pending)
                pending = (bi, t0, TB, A1, B1, packed, C1, C2)

                fire_chunks(bi, fire_at.get(bi, 0))
                if bi == 0:
                    # span-phase constants aren't needed until bi>=5; loading
                    # them here keeps the startup DMA queue clear
                    nc.sync.dma_start(idx2_sb[:], idxd[:, :])
                    nc.scalar.dma_start(mhs_sb[:], mhd[:, :])

            export_block(*pending)
            fire_chunks(len(TOK_BLOCKS), len(chunk_list))

    nc.compile()
    return nc


def _prep_shared(inputs):
    """Host-side weight packing into one [128, WTOT] bf16 wall."""
    f32 = lambda x: np.ascontiguousarray(np.asarray(x), dtype=np.float32)
    aw1, ab1 = f32(inputs["aw1"]), f32(inputs["ab1"])
    aw2, ab2 = f32(inputs["aw2"]), f32(inputs["ab2"])
    aw3, ab3 = f32(inputs["aw3"]), f32(inputs["ab3"])
    sw1, sb1 = f32(inputs["sw1"]), f32(inputs["sb1"])
    sw2, sb2 = f32(inputs["sw2"]), f32(inputs["sb2"])
    sw3, sb3 = f32(inputs["sw3"]), f32(inputs["sb3"])
    wt = f32(inputs["width_table"])

    sw1a, sw1b, sw1p, sw1w = sw1[0:400], sw1[400:800], sw1[800:1150], sw1[1150:1170]

    wl = np.zeros((DS, 96), np.float32)
    wl[:, 0:22] = aw1[:, 128:150]
    wl[:, 32:54] = sw1a[:, 128:150]
    wl[:, 64:86] = sw1b[:, 128:150]

    wall = np.zeros((128, WTOT), BF16NP)
    wk4 = np.concatenate([aw1[:, 0:128], sw1a[:, 0:128], sw1b[:, 0:128], wl], axis=1)
    for i, (k0, k1) in enumerate(K400):
        wall[0:k1 - k0, WOFF[i]:WOFF[i] + 480] = wk4[k0:k1].astype(BF16NP)
    for i, (k0, k1) in enumerate(K350):
        wall[0:k1 - k0, WOFF[4 + i]:WOFF[4 + i] + 150] = sw1p[k0:k1].astype(BF16NP)
    wk1 = np.concatenate([aw2, sw2, aw3, sw3], axis=1)
    for i, (k0, k1) in enumerate(K150):
        wall[0:k1 - k0, WOFF[7 + i]:WOFF[7 + i] + 302] = wk1[k0:k1].astype(BF16NP)

    # width-bin difference table with sb1 folded in (widths are 1..10 -> bin 1..5)
    Wmb = wt @ sw1w  # [9, 150]
    dtab = np.zeros((5, H), np.float32)
    dtab[0] = Wmb[1] + sb1
    for jj in range(1, 5):
        dtab[jj] = Wmb[jj + 1] - Wmb[jj]
    wall[0:5, WOFF[9]:WOFF[9] + 150] = dtab.astype(BF16NP)

    b1p = np.zeros((128, 8), np.float32)
    b1p[:, 0] = ab1[0:128]
    b1p[0:22, 1] = ab1[128:150]
    b1p[:, 2] = ab2[0:128]
    b1p[0:22, 3] = ab2[128:150]
    b1p[:, 4] = sb2[0:128]
    b1p[0:22, 5] = sb2[128:150]
    b1p[0, 6] = sb3[0]
    b1p[0, 7] = ab3[0]
    wall[:, WOFF[10]:WOFF[10] + 16] = b1p.view(BF16NP)
    return wall


def _wrap16(a):
    """int16 index array in the SWDGE layout: idx i at [i%16, i//16],
    replicated 8x across the 128 partitions (one copy per Q7 core)."""
    w = np.ascontiguousarray(a.astype(np.int16).reshape(-1, 16).T)
    return np.ascontiguousarray(np.tile(w, (8, 1)))


def prepare_in_maps(inputs):
    """Host-side sharding: returns (in_maps, sels) — per-core input dicts and
    the original span indices each core's padded slots map back to."""
    states = np.asarray(inputs["states"], dtype=np.float32)
    embeds = np.asarray(inputs["embeds"], dtype=np.float32)
    starts = np.asarray(inputs["span_starts"]).astype(np.int64)
    widths = np.asarray(inputs["span_widths"]).astype(np.int64)

    wall = _prep_shared(inputs)

    bucket = np.minimum(starts // TPC, NCORES - 1)
    order = np.argsort(bucket, kind="stable")
    counts = np.bincount(bucket, minlength=NCORES)
    assert counts.max() <= SMAX, f"span bucket overflow: {counts.max()} > {SMAX}"
    offs = np.zeros(NCORES + 1, np.int64)
    offs[1:] = np.cumsum(counts)

    mh_full = (widths[None, :] >= BINS5[:, None]).astype(np.float32)  # [5, S]

    stage_caps = [NT1 * 128, NT2 * 128, NT3 * 128]
    stage_offs = [0, OFF2, OFF3]
    slot0 = [0, NT1 * 128, (NT1 + NT2) * 128]

    in_maps = []
    sels = []
    for cix in range(NCORES):
        t0 = cix * TPC
        tl = min(T, t0 + TPC + W_MAX - 1) - t0
        seT = np.zeros((896, TL_PAD), BF16NP)
        seT[0:DS, :tl] = states[t0:t0 + tl].T.astype(BF16NP)
        seT[512:512 + DE, :tl] = embeds[t0:t0 + tl].T.astype(BF16NP)

        sel = order[offs[cix]:offs[cix + 1]]
        lsr = (starts[sel] - t0).astype(np.int32)
        ler = lsr + widths[sel].astype(np.int32) - 1

        slot_orig = np.full(SMAX, -1, np.int64)
        ls = np.zeros(SMAX, np.int32)
        le = np.zeros(SMAX, np.int32)
        mh = np.zeros((5, SMAX), BF16NP)
        stg = np.where(ler < SPLIT1, 0, np.where(ler < SPLIT2, 1, 2))
        for s in range(3):
            ix = np.nonzero(stg == s)[0]
            assert len(ix) <= stage_caps[s], f"stage {s} overflow: {len(ix)}"
            base, off = slot0[s], stage_offs[s]
            slot_orig[base:base + len(ix)] = sel[ix]
            # local indices for this stage's table; pad slots keep
            # ls=0 / le=0 (-> gather rows 0 and 1, both finite)
            ls[base:base + len(ix)] = lsr[ix] - off
            le[base:base + len(ix)] = ler[ix] - off
            mh[:, base:base + len(ix)] = mh_full[:, sel[ix]].astype(BF16NP)
        sels.append(slot_orig)

        in_maps.append({
            "seT": seT,
            "idx2": np.concatenate([_wrap16(ls), _wrap16(le + 1)], axis=1),
            "mh": mh,
            "wall": wall,
        })
    return in_maps, sels


def kernel(**inputs) -> np.ndarray:
    in_maps, sels = prepare_in_maps(inputs)

    if "nc" not in _PROGRAM_CACHE:
        _PROGRAM_CACHE["nc"] = _build_program()
    nc = _PROGRAM_CACHE["nc"]

    from concourse.bass_utils import run_bass_kernel_spmd
    res = run_bass_kernel_spmd(nc, in_maps, core_ids=list(range(NCORES)))
    _PROGRAM_CACHE["last_res"] = res  # exec_time_ns etc, for the test harness

    out = np.zeros(S, np.float32)
    for cix in range(NCORES):
        slot_orig = sels[cix]
        m = slot_orig >= 0
        vals = np.asarray(res.results[cix]["scores"]).reshape(-1)
        out[slot_orig[m]] = vals[m]
    return out
